# revision 34
# baseline (speedup 1.0000x reference)
"""DeepGCN (3-layer GCNConv + BN + ReLU) on 8 Trainium2 NeuronCores.

Strategy (graph/data parallel, dst-sharded):
 - Nodes padded to NPAD=50176 = 8 cores x 6272 rows = 392 blocks of 128.
 - Edges partitioned by destination core/block.  Self-loops are NOT in the
   edge streams: the self term is added per dst block with one identity
   matmul of the block's own (pre-scaled) feature rows, read back from the
   layer's shard in DRAM -- saves ~6% of SWDGE descriptor-generation time,
   the kernel's bottleneck engine.
 - Per layer: sharded feature matmul -> AllGather (DRAM) -> copy into an
   SBUF-resident gather table laid out [128 part, 392 rows, H feats] with
   node n at partition n//392, row n%392, so the table fill is 128 fully
   contiguous descriptors instead of 50k 256B ones.
 - Per-dst-block message passing:
     gather source rows with SBUF-source dma_gather (src_is_sbuf ucode
     path; no HBM random-read penalty) ->
     segment-sum via PE matmul with a 0/1 selection matrix built on DVE
     (is_equal of dst_local against an iota row) accumulated in PSUM ->
     ACT epilogue (Relu/scale by D^-1/2, BN affine folded into weights).
 - The layer-2 feature matmul (a1 @ W2s) is fused into mp1's epilogue so
   the AllGather for layer 2 can start the moment mp1 ends.
 - Normalization dis[src]*dis[dst] is separable: pre-scale the table
   rows by dis, post-scale the segment sum by dis.
 - BN: y = s*conv + t with s,t folded into W (columns) and a rank-1 bias
   matmul (invdis x Q) so that dis*(segsum + invdis*Q) = dis*segsum + Q.
 - int16 gather indices (max 32767): idx g = (n%392)*128 + n//392; edges
   split into an A call (n%392 < 256, base 0) and a B call (n%392 >= 136,
   base 17408); the middle band goes to whichever side has slots.
 - Layers 2/3 use 64-wide shards/AllGathers/tables (H2=64).
"""

import numpy as np
import ml_dtypes

import concourse.bacc as bacc
import concourse.mybir as mybir
import concourse.tile as tile
from concourse.bass_utils import run_bass_kernel_spmd

BF16 = ml_dtypes.bfloat16

N = 50000
DIN = 512
H1 = 128
H2 = 64
NCLS = 2
EPS = 1e-5

NCORES = 8
P = 128
NB = 49                  # dst blocks per core
SHN = NB * P             # 6272 nodes per core
NPAD = NCORES * SHN      # 50176
RPP = NPAD // P          # 392 table rows per partition
GRPB = 7                 # blocks per epilogue/m-phase group
NGRP = NB // GRPB        # 7
SGB = 3                  # blocks per gather sub-group (A call + B call each)
SUBGROUPS = [list(range(i, min(i + SGB, NB))) for i in range(0, NB, SGB)]
BBASE = 17408            # gather-idx base of the B window (136*128)
BROW = 136               # table-row offset of the B window
AROW = 256               # A window covers table rows [0, 256)
NQ = 4                   # SWDGE queues for dma_gather
MAXC = 12                # chunks per gather call (small calls -> 4-queue
                         # concurrent SWDGE emission)
GBUFS = 3                # gather-buffer ring depth (subgroups in flight)
SINGLE_PACKET = False
NAGC = 2                 # allgather chunks per layer (chunk0 issued early,
                         # overlapped with the producing phase's tail)
CHR = SHN // NAGC        # 3136 shard rows per chunk = 8 x 392 (aligns with
                         # 8 table partitions per core per chunk)
AGHOOK = 14              # subgroup index at which the next layer's AG
                         # chunk0 is issued (its wait is satisfied by then)

_cache: dict = {}


# --------------------------------------------------------------------------
# host-side preprocessing
# --------------------------------------------------------------------------

def _make_schedule(edge_index):
    """Partition edges by (core, block), pick shared per-block chunk counts.

    Returns sched dict:
      kA, kB: [NB] shared chunk counts per block (A / B gather calls)
      per-core padded edge streams: gidx (int16 gather idx), dloc (f32 dst
      local, -1 for padding), laid out sub-group-major:
      [sg0: A-chunks(b0..b2) | B-chunks(b0..b2)] [sg1: ...]
    """
    src = np.asarray(edge_index[0], np.int64)
    dst = np.asarray(edge_index[1], np.int64)

    core = dst // SHN
    blk = (dst % SHN) // P
    dl = (dst % P).astype(np.int32)
    key = (core * NB + blk).astype(np.int64)
    order = np.argsort(key, kind="stable")
    s_src = src[order]
    s_dl = dl[order]
    s_key = key[order]
    bounds = np.searchsorted(s_key, np.arange(NCORES * NB + 1))

    nlow = np.zeros((NCORES, NB), np.int64)
    nhigh = np.zeros((NCORES, NB), np.int64)
    ntot = np.zeros((NCORES, NB), np.int64)
    segs = {}
    for c in range(NCORES):
        for b in range(NB):
            i0, i1 = bounds[c * NB + b], bounds[c * NB + b + 1]
            ss = s_src[i0:i1]
            dd = s_dl[i0:i1]
            segs[(c, b)] = (ss, dd)
            ntot[c, b] = i1 - i0
            rr = ss % RPP
            nlow[c, b] = int((rr < BROW).sum())
            nhigh[c, b] = int((rr >= AROW).sum())

    K = np.maximum(1, -(-ntot.max(axis=0) // P))          # ceil
    kA_min = -(-nlow.max(axis=0) // P)
    kA_max = K - (-(-nhigh.max(axis=0) // P))
    assert (kA_min <= kA_max).all(), "A/B split infeasible"
    kA = np.clip((K * 2) // 3, kA_min, kA_max)
    kB = K - kA

    gidx_cores, dloc_cores = [], []
    for c in range(NCORES):
        gparts, dparts = [], []
        for sg in SUBGROUPS:
            for side in (0, 1):
                for b in sg:
                    ss, dd = segs[(c, b)]
                    rr = ss % RPP
                    # table partition = (local_row//392)*8 + core: each AG
                    # chunk k fills the contiguous partition slab [64k,64k+64)
                    gg = rr * P + ((ss % SHN) // RPP) * NCORES + ss // SHN
                    low = rr < BROW
                    high = rr >= AROW
                    mid = ~low & ~high
                    slots_a = int(kA[b]) * P
                    mid_idx = np.nonzero(mid)[0]
                    a_take = min(len(mid_idx), slots_a - int(low.sum()))
                    assert a_take >= 0
                    a_sel = np.concatenate([np.nonzero(low)[0], mid_idx[:a_take]])
                    b_sel = np.concatenate([mid_idx[a_take:], np.nonzero(high)[0]])
                    assert len(b_sel) <= int(kB[b]) * P
                    if side == 0:
                        sel, slots, base = a_sel, slots_a, 0
                    else:
                        sel, slots, base = b_sel, int(kB[b]) * P, BBASE
                    idx = gg[sel] - base
                    dloc = dd[sel].astype(np.float32)
                    padn = slots - len(sel)
                    idx = np.concatenate([idx, np.zeros(padn, np.int64)])
                    dloc = np.concatenate([dloc, -np.ones(padn, np.float32)])
                    assert (idx >= 0).all() and (idx < 32768).all()
                    gparts.append(idx.astype(np.int16))
                    dparts.append(dloc)
        gidx_cores.append(np.concatenate(gparts))
        dloc_cores.append(np.concatenate(dparts))

    T = int(K.sum())
    return {
        "kA": kA.astype(int).tolist(),
        "kB": kB.astype(int).tolist(),
        "T": T,
        "gidx": gidx_cores,
        "dloc": dloc_cores,
    }


def _prep_inputs(sched, x, w1, b1, g1, beta1, m1, v1,
                 w2, b2, g2, beta2, m2, v2, w3, b3):
    s1 = g1 / np.sqrt(v1 + EPS)
    t1 = beta1 - m1 * s1
    s2 = g2 / np.sqrt(v2 + EPS)
    t2 = beta2 - m2 * s2
    q1 = (s1 * b1 + t1).astype(np.float32)[None, :]
    q2 = (s2 * b2 + t2).astype(np.float32)[None, :]
    w1s = (w1 * s1[None, :]).astype(np.float32)
    w2s = (w2 * s2[None, :]).astype(np.float32)

    deg = np.zeros(NPAD, np.float32)
    cnt = np.bincount(sched["dst_all"], minlength=N).astype(np.float32)
    deg[:N] = cnt
    dis = np.where(deg > 0, 1.0 / np.sqrt(np.maximum(deg, 1e-30)), 0.0)
    invdis = np.sqrt(deg)

    xp = np.zeros((NPAD, DIN), np.float32)
    xp[:N] = x

    T = sched["T"]
    iota = np.broadcast_to(np.arange(P, dtype=np.float32), (P, P))

    common = {
        "w1s": np.ascontiguousarray(
            w1s.reshape(4, P, H1).transpose(1, 0, 2)).astype(BF16),
        "w2s": w2s.astype(BF16),
        "w3": w3.astype(np.float32),
        "q1": q1.astype(BF16),
        "q2": q2.astype(BF16),
        "b3r": b3.astype(np.float32)[None, :],
        "ones": np.ones((1, P), np.float32),
        "iota": iota.astype(BF16),
    }

    in_maps = []
    for c in range(NCORES):
        off = c * SHN
        xc = xp[off:off + SHN].reshape(NB, P, 4, P)      # [b, n, t, p]
        xtt = np.ascontiguousarray(xc.transpose(3, 0, 2, 1)).astype(BF16)
        disc = np.ascontiguousarray(dis[off:off + SHN].reshape(NB, P).T)
        ivd = np.ascontiguousarray(invdis[off:off + SHN])[None, :].astype(BF16)
        gidx = sched["gidx"][c]
        dloc = sched["dloc"][c]
        idx_sb = np.tile(gidx.reshape(T * 8, 16).T, (8, 1))
        dstl = np.ascontiguousarray(dloc.reshape(T, P).T).astype(BF16)
        m = dict(common)
        m.update({
            "xtt": xtt,
            "disc": disc,
            "ivd": ivd,
            "idx": np.ascontiguousarray(idx_sb),
            "dstl": dstl,
        })
        in_maps.append(m)
    return in_maps


# --------------------------------------------------------------------------
# bass program
# --------------------------------------------------------------------------

def _dma_gather_sbuf(nc, out_ap, in_ap, idxs_ap, num_idxs, elem_size,
                     row_bytes, queue_num, single_packet=SINGLE_PACKET):
    """SBUF-source dma_gather, transpose=False (ucode supports it; the
    public builder only allows transpose=True for SBUF sources).

    Table layout: idx i reads partition i%128, byte range
    [ (i//128)*row_bytes, +elem_bytes ) starting at in_ap's base address.
    Output is edge-major [128, num_idxs//128, elem_size] like the HBM path.
    """
    gp = nc.gpsimd
    inst = gp.add_instruction(
        mybir.InstDMAGatherAnt(
            name=nc.get_next_instruction_name(),
            ins=[
                gp.lower_ap(in_ap),
                gp.lower_ap(idxs_ap),
                gp.lower_val_access(gp.to_reg(num_idxs)),
            ],
            outs=[gp.lower_ap(out_ap)],
            transpose=False,
            num_idxs=num_idxs,
            elem_size=elem_size,
            stride_bytes_256=0,
            gen_mode=0,
            single_packet=single_packet,
            queue_num=queue_num,
            sbuf_tokens_per_rank=P,
            sbuf_free_dim_per_rank=row_bytes,
            sbuf_free_dim_pad_per_rank=0,
            sbuf_byte_offset=0,
        )
    )
    return inst


def _build(sched, stages=99):
    dt = mybir.dt
    kA, kB, T = sched["kA"], sched["kB"], sched["T"]
    K = [a + b for a, b in zip(kA, kB)]

    nc = bacc.Bacc("TRN2", target_bir_lowering=False, debug=False,
                   num_devices=NCORES, num_swdge_queues=NQ,
                   dynamic_dma_scratch_size=8192)

    xtt = nc.dram_tensor("xtt", [P, NB, 4, P], dt.bfloat16, kind="ExternalInput")
    w1s = nc.dram_tensor("w1s", [P, 4, H1], dt.bfloat16, kind="ExternalInput")
    w2s = nc.dram_tensor("w2s", [P, H2], dt.bfloat16, kind="ExternalInput")
    w3 = nc.dram_tensor("w3", [H2, NCLS], dt.float32, kind="ExternalInput")
    q1 = nc.dram_tensor("q1", [1, H1], dt.bfloat16, kind="ExternalInput")
    q2 = nc.dram_tensor("q2", [1, H2], dt.bfloat16, kind="ExternalInput")
    b3r = nc.dram_tensor("b3r", [1, NCLS], dt.float32, kind="ExternalInput")
    ones = nc.dram_tensor("ones", [1, P], dt.float32, kind="ExternalInput")
    iota = nc.dram_tensor("iota", [P, P], dt.bfloat16, kind="ExternalInput")
    disc = nc.dram_tensor("disc", [P, NB], dt.float32, kind="ExternalInput")
    ivd = nc.dram_tensor("ivd", [1, SHN], dt.bfloat16, kind="ExternalInput")
    dstl = nc.dram_tensor("dstl", [P, T], dt.bfloat16, kind="ExternalInput")
    idx = nc.dram_tensor("idx", [P, T * 8], dt.int16, kind="ExternalInput")
    outt = nc.dram_tensor("out", [P, NB * NCLS], dt.float32,
                          kind="ExternalOutput")

    Relu = mybir.ActivationFunctionType.Relu
    Copy = mybir.ActivationFunctionType.Copy
    rg = [list(range(NCORES))]

    with tile.TileContext(nc) as tc:
        with (
            tc.tile_pool(name="cst", bufs=1) as cst,
            tc.tile_pool(name="res", bufs=1) as res,
            tc.tile_pool(name="tbl", bufs=1) as tblp,
            tc.tile_pool(name="dram", bufs=1, space="DRAM") as dram,
            tc.tile_pool(name="work", bufs=2) as work,
            tc.tile_pool(name="sf", bufs=2) as sfp,
            tc.tile_pool(name="gAB", bufs=GBUFS) as gpool,
            tc.tile_pool(name="sp", bufs=2) as spool,
            tc.tile_pool(name="ps", bufs=2, space="PSUM") as pp,
        ):
            # ---- constants into SBUF ----
            def cload(ap_dram, shape, dtype, tag):
                t = cst.tile(shape, dtype, tag=tag)
                nc.sync.dma_start(out=t[:], in_=ap_dram)
                return t

            w1_t = cload(w1s[:], [P, 4, H1], dt.bfloat16, "w1")
            w2_t = cload(w2s[:], [P, H2], dt.bfloat16, "w2")
            w3_t = cload(w3[:], [H2, NCLS], dt.float32, "w3")
            q1_t = cload(q1[:], [1, H1], dt.bfloat16, "q1")
            q2_t = cload(q2[:], [1, H2], dt.bfloat16, "q2")
            b3_t = cload(b3r[:], [1, NCLS], dt.float32, "b3")
            on_t = cload(ones[:], [1, P], dt.float32, "on")
            io_t = cload(iota[:], [P, P], dt.bfloat16, "io")
            di_t = cload(disc[:], [P, NB], dt.float32, "di")
            iv_t = cload(ivd[:], [1, SHN], dt.bfloat16, "iv")
            dl_t = cload(dstl[:], [P, T], dt.bfloat16, "dl")
            ix_t = cload(idx[:], [P, T * 8], dt.int16, "ix")
            from concourse.masks import make_identity
            idn_f = cst.tile([P, P], dt.float32, tag="idf")
            make_identity(nc, idn_f[:])
            idn_b = cst.tile([P, P], dt.bfloat16, tag="idb")
            make_identity(nc, idn_b[:])

            io3 = io_t[:].rearrange("p (a q) -> p a q", a=1)

            # persistent tiles
            A1 = res.tile([P, NB * P], dt.bfloat16, tag="a1")
            outacc = res.tile([P, NB * NCLS], dt.float32, tag="oa")

            # SBUF gather table: one [P, 392*128] bf16 tile (100 KiB/part).
            # mp1 uses it as [P, 392, 128]; mp2/mp3 carve it into two
            # [P, 392, 64] halves (bytes 0..50175 / 50176..100351).
            tbl = tblp.tile([P, RPP * P], dt.bfloat16, tag="tb")
            tb1v = tbl[:].rearrange("p (b h) -> p b h", h=P)
            tb64 = tbl[:].rearrange("p (b h) -> p b h", h=H2)   # [P, 784, 64]

            # dram staging
            shard1 = dram.tile([SHN, P], dt.bfloat16, tag="sh1", name="sh1")
            shard64 = [dram.tile([SHN, H2], dt.bfloat16, tag=f"s64_{i}",
                                 name=f"s64_{i}") for i in range(2)]
            # chunk-major: per chunk [core, 3136 rows, H] -- both the AG
            # chunk APs and the per-chunk table fills stay fully contiguous.
            # One Shared tensor per chunk (Shared DRAM allows one writer).
            full1 = [dram.tile([NCORES, CHR, P], dt.bfloat16,
                               addr_space="Shared", tag=f"f1_{k}",
                               name=f"f1_{k}") for k in range(NAGC)]
            full64 = [[dram.tile([NCORES, CHR, H2], dt.bfloat16,
                                 addr_space="Shared", tag=f"f64_{i}_{k}",
                                 name=f"f64_{i}_{k}") for k in range(NAGC)]
                      for i in range(2)]

            def allgather(si, fi, k):
                nc.gpsimd.collective_compute(
                    "AllGather", mybir.AluOpType.bypass, replica_groups=rg,
                    ins=[si[k * CHR:(k + 1) * CHR, :]], outs=[fi[k][:]],
                )

            # table partition p = q*8 + c (q = local_row//392, c = core)
            # holds ranks 0..392; chunk k covers partitions [64k, 64k+64):
            # a plain contiguous partition slice on the SBUF side, with the
            # (q c) interleave expressed on the DRAM side.
            def fill_chunk(fi, tv, row0, k):
                for qq in range(8):
                    nc.sync.dma_start(
                        out=tv[64 * k + 8 * qq:64 * k + 8 * qq + 8,
                               row0:row0 + RPP, :],
                        in_=fi[k][:, qq * RPP:(qq + 1) * RPP, :])

            nc.vector.memset(outacc[:], 0.0)

            # ---- phase M1: h1 = dis * (x @ W1s) -> shard1 ----
            with nc.named_scope("m1"):
                for g in range(NGRP):
                    xg = work.tile([P, GRPB, 4, P], dt.bfloat16, tag="xg")
                    nc.sync.dma_start(
                        out=xg[:], in_=xtt[:, g * GRPB:(g + 1) * GRPB, :, :])
                    hb = work.tile([P, GRPB, H1], dt.bfloat16, tag="hb")
                    for j in range(GRPB):
                        b = g * GRPB + j
                        ps = pp.tile([P, H1], dt.float32, tag="mp")
                        for t in range(4):
                            nc.tensor.matmul(ps[:], xg[:, j, t, :],
                                             w1_t[:, t, :],
                                             start=(t == 0), stop=(t == 3))
                        nc.scalar.activation(hb[:, j, :], ps[:], Copy,
                                             scale=di_t[:, b:b + 1])
                    nc.sync.dma_start(
                        out=shard1[g * GRPB * P:(g + 1) * GRPB * P, :]
                        .rearrange("(b p) h -> p b h", p=P),
                        in_=hb[:])
                    if g == 3 and stages >= 2:
                        # rows [0, 3136) are written after group 3
                        allgather(shard1, full1, 0)
                if stages >= 2:
                    allgather(shard1, full1, 1)
                    fill_chunk(full1, tb1v, 0, 0)
                    fill_chunk(full1, tb1v, 0, 1)

            # ---- message-passing layer ----
            self_qn = [0]
            capA = max(sum(kA[b] for b in sg) for sg in SUBGROUPS)
            capB = max(sum(kB[b] for b in sg) for sg in SUBGROUPS)

            def mp_layer(lname, tview, rank0, H, row_bytes, qrow, selfsrc,
                         epilogue, hook=None):
                """tview: [P, rows, H] table view; rank0: row offset of the
                view's base (for the B window slice).  selfsrc: DRAM shard
                holding this core's own (pre-scaled) rows; one identity
                matmul per dst block adds the self-loop term.  Gathers are
                split into calls of <= MAXC chunks, round-robin over the 4
                SWDGE queues (the 4 Q7 core pairs emit concurrently)."""
                gcol = 0
                inA = tview[:, rank0:rank0 + AROW, :]
                inB = tview[:, rank0 + BROW:rank0 + RPP, :]
                with nc.named_scope(lname):
                    for sgi, sg in enumerate(SUBGROUPS):
                        if hook is not None and sgi == AGHOOK:
                            hook()
                        nsgb = len(sg)
                        sf = sfp.tile([P, SGB, H1], dt.bfloat16, tag="sf")
                        nc.sync.dma_start(
                            out=sf[:, 0:nsgb, 0:H],
                            in_=selfsrc[sg[0] * P:(sg[-1] + 1) * P, :]
                            .rearrange("(b p) h -> p b h", p=P))
                        KAg = sum(kA[b] for b in sg)
                        KBg = sum(kB[b] for b in sg)
                        gab = gpool.tile([P, (capA + capB) * P], dt.bfloat16,
                                         tag="gab")
                        gva = gvb = None
                        if KAg:
                            gva = gab[:, 0:KAg * H].rearrange(
                                "p (c h) -> p c h", h=H)
                            for c0 in range(0, KAg, MAXC):
                                nch = min(MAXC, KAg - c0)
                                o = gcol + c0
                                _dma_gather_sbuf(
                                    nc, gva[:, c0:c0 + nch, :], inA,
                                    ix_t[:, o * 8:(o + nch) * 8],
                                    nch * P, H, row_bytes, self_qn[0] % NQ)
                                self_qn[0] += 1
                        if KBg:
                            gvb = gab[:, KAg * H:(KAg + KBg) * H].rearrange(
                                "p (c h) -> p c h", h=H)
                            for c0 in range(0, KBg, MAXC):
                                nch = min(MAXC, KBg - c0)
                                o = gcol + KAg + c0
                                _dma_gather_sbuf(
                                    nc, gvb[:, c0:c0 + nch, :], inB,
                                    ix_t[:, o * 8:(o + nch) * 8],
                                    nch * P, H, row_bytes, self_qn[0] % NQ)
                                self_qn[0] += 1
                        # per-block segment sums
                        aoff = 0
                        boff = 0
                        for bj, b in enumerate(sg):
                            ka, kb = kA[b], kB[b]
                            S = spool.tile([P, (ka + kb) * P], dt.float8e4,
                                           tag="s")
                            s3 = S[:].rearrange("p (c q) -> p c q", q=P)
                            ca = gcol + aoff
                            cb = gcol + KAg + boff
                            if ka:
                                nc.vector.tensor_tensor(
                                    s3[:, 0:ka, :],
                                    dl_t[:, ca:ca + ka].to_broadcast([P, ka, P]),
                                    io3.to_broadcast([P, ka, P]),
                                    op=mybir.AluOpType.is_equal)
                            if kb:
                                nc.vector.tensor_tensor(
                                    s3[:, ka:ka + kb, :],
                                    dl_t[:, cb:cb + kb].to_broadcast([P, kb, P]),
                                    io3.to_broadcast([P, kb, P]),
                                    op=mybir.AluOpType.is_equal)
                            ps = pp.tile([P, H1], dt.float32, tag="mp")
                            psv = ps[:, 0:H]
                            first = True
                            if qrow is not None:
                                nc.tensor.matmul(
                                    psv,
                                    iv_t[0:1, b * P:(b + 1) * P],
                                    qrow[0:1, :], start=True, stop=False)
                                first = False
                            # self-loop term: psv += I @ sf[b] (= table row)
                            nc.tensor.matmul(
                                psv, idn_b[:], sf[:, bj, 0:H],
                                start=first, stop=False)
                            first = False
                            nch = ka + kb
                            for c in range(ka):
                                nc.tensor.matmul(
                                    psv, s3[:, c, :], gva[:, aoff + c, 0:H],
                                    start=False,
                                    stop=(c == nch - 1))
                            for c in range(kb):
                                nc.tensor.matmul(
                                    psv, s3[:, ka + c, :], gvb[:, boff + c, 0:H],
                                    start=False,
                                    stop=(ka + c == nch - 1))
                            epilogue(b, psv)
                            aoff += ka
                            boff += kb
                        gcol += KAg + KBg

            # ---- epilogues ----
            epi1_st = {}

            def epi1(b, psv):
                g, j = b // GRPB, b % GRPB
                if j == 0:
                    epi1_st["t"] = work.tile([P, GRPB, H2], dt.bfloat16,
                                             tag="h2b", name="h2b")
                h2b = epi1_st["t"]
                tmp = work.tile([P, H1], dt.float32, tag="ep")
                d = di_t[:, b:b + 1]
                nc.scalar.activation(tmp[:], psv, Relu, scale=d)
                nc.scalar.activation(A1[:, b * P:(b + 1) * P], tmp[:], Copy,
                                     scale=d)
                # fused layer-2 feature matmul: h2 = A1_b @ W2s
                pst = pp.tile([P, P], dt.bfloat16, tag="trb")
                nc.tensor.transpose(pst[:], A1[:, b * P:(b + 1) * P], idn_b[:])
                a1T = work.tile([P, P], dt.bfloat16, tag="a1T")
                nc.scalar.activation(a1T[:], pst[:], Copy)
                ps2 = pp.tile([P, H1], dt.float32, tag="mp")
                nc.tensor.matmul(ps2[:, 0:H2], a1T[:], w2_t[:],
                                 start=True, stop=True)
                nc.scalar.activation(h2b[:, j, :], ps2[:, 0:H2], Copy)
                if j == GRPB - 1:
                    nc.sync.dma_start(
                        out=shard64[0][g * GRPB * P:(g + 1) * GRPB * P, :]
                        .rearrange("(b p) h -> p b h", p=P),
                        in_=h2b[:])

            epi2_st = {}

            def epi2(b, psv):
                g, j = b // GRPB, b % GRPB
                if j == 0:
                    epi2_st["t"] = work.tile([P, GRPB, H2], dt.bfloat16,
                                             tag="a2b", name="a2b")
                a2b = epi2_st["t"]
                tmp = work.tile([P, H2], dt.float32, tag="ep")
                d = di_t[:, b:b + 1]
                nc.scalar.activation(tmp[:], psv, Relu, scale=d)
                nc.scalar.activation(a2b[:, j, :], tmp[:], Copy, scale=d)
                if j == GRPB - 1:
                    nc.sync.dma_start(
                        out=shard64[1][g * GRPB * P:(g + 1) * GRPB * P, :]
                        .rearrange("(b p) h -> p b h", p=P),
                        in_=a2b[:])

            def epi3(b, psv):
                r = work.tile([P, H2], dt.float32, tag="ep")
                nc.scalar.activation(r[:], psv, Copy, scale=di_t[:, b:b + 1])
                pst = pp.tile([H2, P], dt.float32, tag="tr")
                nc.tensor.transpose(pst[:], r[:], idn_f[:])
                rT = work.tile([H2, P], dt.float32, tag="rT")
                nc.scalar.activation(rT[:], pst[:], Copy)
                ps3 = pp.tile([P, NCLS], dt.float32, tag="o3")
                nc.tensor.matmul(ps3[:], rT[:], w3_t[:], start=True, stop=False)
                nc.tensor.matmul(ps3[:], on_t[0:1, :], b3_t[0:1, :],
                                 start=False, stop=True)
                nc.scalar.activation(outacc[:, b * NCLS:(b + 1) * NCLS],
                                     ps3[:], Copy)

            if stages >= 3:
                # AG2 chunk0 is issued from inside mp1 (hook): by subgroup
                # AGHOOK the first 28 blocks' h2 shard rows have landed, so
                # the collective's wait costs the Pool queue nothing and the
                # transfer overlaps mp1's tail.
                mp_layer("mp1", tb1v, 0, H1, 256, q1_t, shard1, epi1,
                         hook=(lambda: allgather(shard64[0], full64[0], 0))
                         if stages >= 5 else None)

            if stages >= 5:
                with nc.named_scope("ag2"):
                    allgather(shard64[0], full64[0], 1)
                    fill_chunk(full64[0], tb64, 0, 0)
                    fill_chunk(full64[0], tb64, 0, 1)

            if stages >= 6:
                mp_layer("mp2", tb64, 0, H2, H2 * 2, q2_t, shard64[0], epi2,
                         hook=(lambda: allgather(shard64[1], full64[1], 0))
                         if stages >= 7 else None)
            if stages >= 7:
                with nc.named_scope("ag3"):
                    allgather(shard64[1], full64[1], 1)
                    fill_chunk(full64[1], tb64, RPP, 0)
                    fill_chunk(full64[1], tb64, RPP, 1)
            if stages >= 8:
                mp_layer("mp3", tb64, RPP, H2, H2 * 2, None, shard64[1], epi3)

            with nc.named_scope("fin"):
                nc.sync.dma_start(out=outt.ap(), in_=outacc[:])

    nc.compile()
    return nc


def _run(inputs, trace=False, stages=99):
    x = np.asarray(inputs["x"], np.float32)
    edge_index = np.asarray(inputs["edge_index"])
    key = (hash(edge_index.tobytes()), stages)
    if key not in _cache:
        sched = _make_schedule(edge_index)
        sched["dst_all"] = np.concatenate(
            [edge_index[1], np.arange(N, dtype=np.int64)]).astype(np.int64)
        nc = _build(sched, stages=stages)
        _cache[key] = (sched, nc)
    sched, nc = _cache[key]
    sched["dst_all"] = np.concatenate(
        [edge_index[1], np.arange(N, dtype=np.int64)]).astype(np.int64)

    in_maps = _prep_inputs(
        sched, x,
        np.asarray(inputs["w1"], np.float32), np.asarray(inputs["b1"], np.float32),
        np.asarray(inputs["g1"], np.float32), np.asarray(inputs["beta1"], np.float32),
        np.asarray(inputs["m1"], np.float32), np.asarray(inputs["v1"], np.float32),
        np.asarray(inputs["w2"], np.float32), np.asarray(inputs["b2"], np.float32),
        np.asarray(inputs["g2"], np.float32), np.asarray(inputs["beta2"], np.float32),
        np.asarray(inputs["m2"], np.float32), np.asarray(inputs["v2"], np.float32),
        np.asarray(inputs["w3"], np.float32), np.asarray(inputs["b3"], np.float32),
    )
    kw = {}
    if trace:
        kw = dict(trace=True, trace_cores=list(range(NCORES)))
    res = run_bass_kernel_spmd(nc, in_maps, core_ids=list(range(NCORES)), **kw)
    outs = []
    for c in range(NCORES):
        o = np.asarray(res.results[c]["out"])          # [P, NB*NCLS]
        outs.append(o.reshape(P, NB, NCLS).transpose(1, 0, 2).reshape(SHN, NCLS))
    out = np.concatenate(outs, axis=0)
    return out[:N].astype(np.float32), res


def kernel(**inputs) -> np.ndarray:
    out, _ = _run(inputs, trace=False)
    return out


# revision 36
# speedup vs baseline: 1.0402x; 1.0402x over previous
"""DeepGCN (3-layer GCNConv + BN + ReLU) on 8 Trainium2 NeuronCores.

Strategy (graph/data parallel, dst-sharded):
 - Nodes padded to NPAD=50176 = 8 cores x 6272 rows = 392 blocks of 128.
 - Edges partitioned by destination core/block.  Self-loops are NOT in the
   edge streams: the self term is added per dst block with one identity
   matmul of the block's own (pre-scaled) feature rows, read back from the
   layer's shard in DRAM -- saves ~6% of SWDGE descriptor-generation time,
   the kernel's bottleneck engine.
 - Per layer: sharded feature matmul -> AllGather (DRAM) -> copy into an
   SBUF-resident gather table laid out [128 part, 392 rows, H feats] with
   node n at partition n//392, row n%392, so the table fill is 128 fully
   contiguous descriptors instead of 50k 256B ones.
 - Per-dst-block message passing:
     gather source rows with SBUF-source dma_gather (src_is_sbuf ucode
     path; no HBM random-read penalty) ->
     segment-sum via PE matmul with a 0/1 selection matrix built on DVE
     (is_equal of dst_local against an iota row) accumulated in PSUM ->
     ACT epilogue (Relu/scale by D^-1/2, BN affine folded into weights).
 - The layer-2 feature matmul (a1 @ W2s) is fused into mp1's epilogue so
   the AllGather for layer 2 can start the moment mp1 ends.
 - Normalization dis[src]*dis[dst] is separable: pre-scale the table
   rows by dis, post-scale the segment sum by dis.
 - BN: y = s*conv + t with s,t folded into W (columns) and a rank-1 bias
   matmul (invdis x Q) so that dis*(segsum + invdis*Q) = dis*segsum + Q.
 - int16 gather indices (max 32767): idx g = (n%392)*128 + n//392; edges
   split into an A call (n%392 < 256, base 0) and a B call (n%392 >= 136,
   base 17408); the middle band goes to whichever side has slots.
 - Layers 2/3 use 64-wide shards/AllGathers/tables (H2=64).
"""

import numpy as np
import ml_dtypes

import concourse.bacc as bacc
import concourse.mybir as mybir
import concourse.tile as tile
from concourse.bass_utils import run_bass_kernel_spmd

BF16 = ml_dtypes.bfloat16

N = 50000
DIN = 512
H1 = 128
H2 = 64
NCLS = 2
EPS = 1e-5

NCORES = 8
P = 128
NB = 49                  # dst blocks per core
SHN = NB * P             # 6272 nodes per core
NPAD = NCORES * SHN      # 50176
RPP = NPAD // P          # 392 table rows per partition
GRPB = 7                 # blocks per epilogue/m-phase group
NGRP = NB // GRPB        # 7
SGB = 3                  # blocks per gather sub-group (A call + B call each)
SUBGROUPS = [list(range(i, min(i + SGB, NB))) for i in range(0, NB, SGB)]
BBASE = 17408            # gather-idx base of the B window (136*128)
BROW = 136               # table-row offset of the B window
AROW = 256               # A window covers table rows [0, 256)
NQ = 4                   # SWDGE queues for dma_gather
MAXC = 12                # chunks per gather call (small calls -> 4-queue
                         # concurrent SWDGE emission)
GBUFS = 3                # gather-buffer ring depth (subgroups in flight)
SINGLE_PACKET = False
NAGC = 2                 # allgather chunks per layer (chunk0 issued early,
                         # overlapped with the producing phase's tail)
CHR = SHN // NAGC        # 3136 shard rows per chunk = 8 x 392 (aligns with
                         # 8 table partitions per core per chunk)
AGHOOK = 14              # subgroup index at which the next layer's AG
                         # chunk0 is issued (its wait is satisfied by then)

_cache: dict = {}


# --------------------------------------------------------------------------
# host-side preprocessing
# --------------------------------------------------------------------------

def _make_schedule(edge_index):
    """Partition edges by (core, block), pick shared per-block chunk counts.

    Returns sched dict:
      kA, kB: [NB] shared chunk counts per block (A / B gather calls)
      per-core padded edge streams: gidx (int16 gather idx), dloc (f32 dst
      local, -1 for padding), laid out sub-group-major:
      [sg0: A-chunks(b0..b2) | B-chunks(b0..b2)] [sg1: ...]
    """
    src = np.asarray(edge_index[0], np.int64)
    dst = np.asarray(edge_index[1], np.int64)

    core = dst // SHN
    blk = (dst % SHN) // P
    dl = (dst % P).astype(np.int32)
    key = (core * NB + blk).astype(np.int64)
    order = np.argsort(key, kind="stable")
    s_src = src[order]
    s_dl = dl[order]
    s_key = key[order]
    bounds = np.searchsorted(s_key, np.arange(NCORES * NB + 1))

    nlow = np.zeros((NCORES, NB), np.int64)
    nhigh = np.zeros((NCORES, NB), np.int64)
    ntot = np.zeros((NCORES, NB), np.int64)
    segs = {}
    for c in range(NCORES):
        for b in range(NB):
            i0, i1 = bounds[c * NB + b], bounds[c * NB + b + 1]
            ss = s_src[i0:i1]
            dd = s_dl[i0:i1]
            segs[(c, b)] = (ss, dd)
            ntot[c, b] = i1 - i0
            rr = ss % RPP
            nlow[c, b] = int((rr < BROW).sum())
            nhigh[c, b] = int((rr >= AROW).sum())

    K = np.maximum(1, -(-ntot.max(axis=0) // P))          # ceil
    kA_min = -(-nlow.max(axis=0) // P)
    kA_max = K - (-(-nhigh.max(axis=0) // P))
    assert (kA_min <= kA_max).all(), "A/B split infeasible"
    kA = np.clip((K * 2) // 3, kA_min, kA_max)
    kB = K - kA

    gidx_cores, dloc_cores = [], []
    for c in range(NCORES):
        gparts, dparts = [], []
        for sg in SUBGROUPS:
            for side in (0, 1):
                for b in sg:
                    ss, dd = segs[(c, b)]
                    rr = ss % RPP
                    # table partition = (local_row//392)*8 + core
                    gg = rr * P + ((ss % SHN) // RPP) * NCORES + ss // SHN
                    low = rr < BROW
                    high = rr >= AROW
                    mid = ~low & ~high
                    slots_a = int(kA[b]) * P
                    mid_idx = np.nonzero(mid)[0]
                    a_take = min(len(mid_idx), slots_a - int(low.sum()))
                    assert a_take >= 0
                    a_sel = np.concatenate([np.nonzero(low)[0], mid_idx[:a_take]])
                    b_sel = np.concatenate([mid_idx[a_take:], np.nonzero(high)[0]])
                    assert len(b_sel) <= int(kB[b]) * P
                    if side == 0:
                        sel, slots, base = a_sel, slots_a, 0
                    else:
                        sel, slots, base = b_sel, int(kB[b]) * P, BBASE
                    idx = gg[sel] - base
                    dloc = dd[sel].astype(np.float32)
                    padn = slots - len(sel)
                    idx = np.concatenate([idx, np.zeros(padn, np.int64)])
                    dloc = np.concatenate([dloc, -np.ones(padn, np.float32)])
                    assert (idx >= 0).all() and (idx < 32768).all()
                    gparts.append(idx.astype(np.int16))
                    dparts.append(dloc)
        gidx_cores.append(np.concatenate(gparts))
        dloc_cores.append(np.concatenate(dparts))

    T = int(K.sum())
    return {
        "kA": kA.astype(int).tolist(),
        "kB": kB.astype(int).tolist(),
        "T": T,
        "gidx": gidx_cores,
        "dloc": dloc_cores,
    }


def _prep_inputs(sched, x, w1, b1, g1, beta1, m1, v1,
                 w2, b2, g2, beta2, m2, v2, w3, b3):
    s1 = g1 / np.sqrt(v1 + EPS)
    t1 = beta1 - m1 * s1
    s2 = g2 / np.sqrt(v2 + EPS)
    t2 = beta2 - m2 * s2
    q1 = (s1 * b1 + t1).astype(np.float32)[None, :]
    q2 = (s2 * b2 + t2).astype(np.float32)[None, :]
    w1s = (w1 * s1[None, :]).astype(np.float32)
    w2s = (w2 * s2[None, :]).astype(np.float32)

    deg = np.zeros(NPAD, np.float32)
    cnt = np.bincount(sched["dst_all"], minlength=N).astype(np.float32)
    deg[:N] = cnt
    dis = np.where(deg > 0, 1.0 / np.sqrt(np.maximum(deg, 1e-30)), 0.0)
    invdis = np.sqrt(deg)

    xp = np.zeros((NPAD, DIN), np.float32)
    xp[:N] = x

    T = sched["T"]
    iota = np.broadcast_to(np.arange(P, dtype=np.float32), (P, P))

    common = {
        "w1s": np.ascontiguousarray(
            w1s.reshape(4, P, H1).transpose(1, 0, 2)).astype(BF16),
        "w2s": w2s.astype(BF16),
        "w3": w3.astype(np.float32),
        "q1": q1.astype(BF16),
        "q2": q2.astype(BF16),
        "b3r": b3.astype(np.float32)[None, :],
        "ones": np.ones((1, P), np.float32),
        "iota": iota.astype(BF16),
    }

    in_maps = []
    for c in range(NCORES):
        off = c * SHN
        xc = xp[off:off + SHN].reshape(NB, P, 4, P)      # [b, n, t, p]
        xtt = np.ascontiguousarray(xc.transpose(3, 0, 2, 1)).astype(BF16)
        disc = np.ascontiguousarray(dis[off:off + SHN].reshape(NB, P).T)
        ivd = np.ascontiguousarray(invdis[off:off + SHN])[None, :].astype(BF16)
        gidx = sched["gidx"][c]
        dloc = sched["dloc"][c]
        idx_sb = np.tile(gidx.reshape(T * 8, 16).T, (8, 1))
        dstl = np.ascontiguousarray(dloc.reshape(T, P).T).astype(BF16)
        m = dict(common)
        m.update({
            "xtt": xtt,
            "disc": disc,
            "ivd": ivd,
            "idx": np.ascontiguousarray(idx_sb),
            "dstl": dstl,
        })
        in_maps.append(m)
    return in_maps


# --------------------------------------------------------------------------
# bass program
# --------------------------------------------------------------------------

def _dma_gather_sbuf(nc, out_ap, in_ap, idxs_ap, num_idxs, elem_size,
                     row_bytes, queue_num, single_packet=SINGLE_PACKET):
    """SBUF-source dma_gather, transpose=False (ucode supports it; the
    public builder only allows transpose=True for SBUF sources).

    Table layout: idx i reads partition i%128, byte range
    [ (i//128)*row_bytes, +elem_bytes ) starting at in_ap's base address.
    Output is edge-major [128, num_idxs//128, elem_size] like the HBM path.
    """
    gp = nc.gpsimd
    inst = gp.add_instruction(
        mybir.InstDMAGatherAnt(
            name=nc.get_next_instruction_name(),
            ins=[
                gp.lower_ap(in_ap),
                gp.lower_ap(idxs_ap),
                gp.lower_val_access(gp.to_reg(num_idxs)),
            ],
            outs=[gp.lower_ap(out_ap)],
            transpose=False,
            num_idxs=num_idxs,
            elem_size=elem_size,
            stride_bytes_256=0,
            gen_mode=0,
            single_packet=single_packet,
            queue_num=queue_num,
            sbuf_tokens_per_rank=P,
            sbuf_free_dim_per_rank=row_bytes,
            sbuf_free_dim_pad_per_rank=0,
            sbuf_byte_offset=0,
        )
    )
    return inst


def _build(sched, stages=99):
    dt = mybir.dt
    kA, kB, T = sched["kA"], sched["kB"], sched["T"]
    K = [a + b for a, b in zip(kA, kB)]

    nc = bacc.Bacc("TRN2", target_bir_lowering=False, debug=False,
                   num_devices=NCORES, num_swdge_queues=NQ,
                   dynamic_dma_scratch_size=8192)

    xtt = nc.dram_tensor("xtt", [P, NB, 4, P], dt.bfloat16, kind="ExternalInput")
    w1s = nc.dram_tensor("w1s", [P, 4, H1], dt.bfloat16, kind="ExternalInput")
    w2s = nc.dram_tensor("w2s", [P, H2], dt.bfloat16, kind="ExternalInput")
    w3 = nc.dram_tensor("w3", [H2, NCLS], dt.float32, kind="ExternalInput")
    q1 = nc.dram_tensor("q1", [1, H1], dt.bfloat16, kind="ExternalInput")
    q2 = nc.dram_tensor("q2", [1, H2], dt.bfloat16, kind="ExternalInput")
    b3r = nc.dram_tensor("b3r", [1, NCLS], dt.float32, kind="ExternalInput")
    ones = nc.dram_tensor("ones", [1, P], dt.float32, kind="ExternalInput")
    iota = nc.dram_tensor("iota", [P, P], dt.bfloat16, kind="ExternalInput")
    disc = nc.dram_tensor("disc", [P, NB], dt.float32, kind="ExternalInput")
    ivd = nc.dram_tensor("ivd", [1, SHN], dt.bfloat16, kind="ExternalInput")
    dstl = nc.dram_tensor("dstl", [P, T], dt.bfloat16, kind="ExternalInput")
    idx = nc.dram_tensor("idx", [P, T * 8], dt.int16, kind="ExternalInput")
    outt = nc.dram_tensor("out", [P, NB * NCLS], dt.float32,
                          kind="ExternalOutput")

    Relu = mybir.ActivationFunctionType.Relu
    Copy = mybir.ActivationFunctionType.Copy
    rg = [list(range(NCORES))]

    with tile.TileContext(nc) as tc:
        with (
            tc.tile_pool(name="cst", bufs=1) as cst,
            tc.tile_pool(name="res", bufs=1) as res,
            tc.tile_pool(name="tbl", bufs=1) as tblp,
            tc.tile_pool(name="dram", bufs=1, space="DRAM") as dram,
            tc.tile_pool(name="work", bufs=2) as work,
            tc.tile_pool(name="sf", bufs=2) as sfp,
            tc.tile_pool(name="gAB", bufs=GBUFS) as gpool,
            tc.tile_pool(name="sp", bufs=2) as spool,
            tc.tile_pool(name="ps", bufs=2, space="PSUM") as pp,
        ):
            # ---- constants into SBUF ----
            def cload(ap_dram, shape, dtype, tag):
                t = cst.tile(shape, dtype, tag=tag)
                nc.sync.dma_start(out=t[:], in_=ap_dram)
                return t

            w1_t = cload(w1s[:], [P, 4, H1], dt.bfloat16, "w1")
            w2_t = cload(w2s[:], [P, H2], dt.bfloat16, "w2")
            w3_t = cload(w3[:], [H2, NCLS], dt.float32, "w3")
            q1_t = cload(q1[:], [1, H1], dt.bfloat16, "q1")
            q2_t = cload(q2[:], [1, H2], dt.bfloat16, "q2")
            b3_t = cload(b3r[:], [1, NCLS], dt.float32, "b3")
            on_t = cload(ones[:], [1, P], dt.float32, "on")
            io_t = cload(iota[:], [P, P], dt.bfloat16, "io")
            di_t = cload(disc[:], [P, NB], dt.float32, "di")
            iv_t = cload(ivd[:], [1, SHN], dt.bfloat16, "iv")
            dl_t = cload(dstl[:], [P, T], dt.bfloat16, "dl")
            ix_t = cload(idx[:], [P, T * 8], dt.int16, "ix")
            from concourse.masks import make_identity
            idn_f = cst.tile([P, P], dt.float32, tag="idf")
            make_identity(nc, idn_f[:])
            idn_b = cst.tile([P, P], dt.bfloat16, tag="idb")
            make_identity(nc, idn_b[:])

            io3 = io_t[:].rearrange("p (a q) -> p a q", a=1)

            # persistent tiles
            A1 = res.tile([P, NB * P], dt.bfloat16, tag="a1")
            outacc = res.tile([P, NB * NCLS], dt.float32, tag="oa")

            # SBUF gather table: one [P, 392*128] bf16 tile (100 KiB/part).
            # mp1 uses it as [P, 392, 128]; mp2/mp3 carve it into two
            # [P, 392, 64] halves (bytes 0..50175 / 50176..100351).
            tbl = tblp.tile([P, RPP * P], dt.bfloat16, tag="tb")
            tb1v = tbl[:].rearrange("p (b h) -> p b h", h=P)
            tb64 = tbl[:].rearrange("p (b h) -> p b h", h=H2)   # [P, 784, 64]

            # dram staging
            shard1 = dram.tile([SHN, P], dt.bfloat16, tag="sh1", name="sh1")
            shard64 = [dram.tile([SHN, H2], dt.bfloat16, tag=f"s64_{i}",
                                 name=f"s64_{i}") for i in range(2)]
            full1 = [dram.tile([NCORES, CHR, P], dt.bfloat16,
                               addr_space="Shared", tag=f"f1_{k}",
                               name=f"f1_{k}") for k in range(NAGC)]
            full64 = [dram.tile([NPAD, H2], dt.bfloat16, addr_space="Shared",
                                tag=f"f64_{i}", name=f"f64_{i}")
                      for i in range(2)]

            def allgather(si, fi):
                nc.gpsimd.collective_compute(
                    "AllGather", mybir.AluOpType.bypass, replica_groups=rg,
                    ins=[si[:]], outs=[fi[:]],
                )

            def allgather1(si, fi, k):
                nc.gpsimd.collective_compute(
                    "AllGather", mybir.AluOpType.bypass, replica_groups=rg,
                    ins=[si[k * CHR:(k + 1) * CHR, :]], outs=[fi[k][:]],
                )

            # table partition p = q*8 + c (q = local_row//392, c = core).
            # AG1 chunk k covers partitions [64k, 64k+64): fill in 8
            # contiguous-partition slabs (plain APs only on the SBUF side).
            def fill_chunk(fi, tv, row0, k):
                for qq in range(8):
                    nc.sync.dma_start(
                        out=tv[64 * k + 8 * qq:64 * k + 8 * qq + 8,
                               row0:row0 + RPP, :],
                        in_=fi[k][:, qq * RPP:(qq + 1) * RPP, :])

            def fill_full(fi, tv, row0):
                fv = fi[:].rearrange("(c l) h -> c l h", c=NCORES)
                for qq in range(16):
                    nc.sync.dma_start(
                        out=tv[8 * qq:8 * qq + 8, row0:row0 + RPP, :],
                        in_=fv[:, qq * RPP:(qq + 1) * RPP, :])

            nc.vector.memset(outacc[:], 0.0)

            # ---- phase M1: h1 = dis * (x @ W1s) -> shard1 ----
            with nc.named_scope("m1"):
                for g in range(NGRP):
                    xg = work.tile([P, GRPB, 4, P], dt.bfloat16, tag="xg")
                    nc.sync.dma_start(
                        out=xg[:], in_=xtt[:, g * GRPB:(g + 1) * GRPB, :, :])
                    hb = work.tile([P, GRPB, H1], dt.bfloat16, tag="hb")
                    for j in range(GRPB):
                        b = g * GRPB + j
                        ps = pp.tile([P, H1], dt.float32, tag="mp")
                        for t in range(4):
                            nc.tensor.matmul(ps[:], xg[:, j, t, :],
                                             w1_t[:, t, :],
                                             start=(t == 0), stop=(t == 3))
                        nc.scalar.activation(hb[:, j, :], ps[:], Copy,
                                             scale=di_t[:, b:b + 1])
                    nc.sync.dma_start(
                        out=shard1[g * GRPB * P:(g + 1) * GRPB * P, :]
                        .rearrange("(b p) h -> p b h", p=P),
                        in_=hb[:])
                    if g == 3 and stages >= 2:
                        # shard rows [0, 3136) complete after group 3; the
                        # Pool queue is idle during m1 so this wait is free
                        allgather1(shard1, full1, 0)
                if stages >= 2:
                    allgather1(shard1, full1, 1)
                    fill_chunk(full1, tb1v, 0, 0)
                    fill_chunk(full1, tb1v, 0, 1)

            # ---- message-passing layer ----
            self_qn = [0]
            capA = max(sum(kA[b] for b in sg) for sg in SUBGROUPS)
            capB = max(sum(kB[b] for b in sg) for sg in SUBGROUPS)

            def mp_layer(lname, tview, rank0, H, row_bytes, qrow, selfsrc,
                         epilogue, hook=None):
                """tview: [P, rows, H] table view; rank0: row offset of the
                view's base (for the B window slice).  selfsrc: DRAM shard
                holding this core's own (pre-scaled) rows; one identity
                matmul per dst block adds the self-loop term.  Gathers are
                split into calls of <= MAXC chunks, round-robin over the 4
                SWDGE queues (the 4 Q7 core pairs emit concurrently)."""
                gcol = 0
                inA = tview[:, rank0:rank0 + AROW, :]
                inB = tview[:, rank0 + BROW:rank0 + RPP, :]
                with nc.named_scope(lname):
                    for sgi, sg in enumerate(SUBGROUPS):
                        if hook is not None and sgi == AGHOOK:
                            hook()
                        nsgb = len(sg)
                        sf = sfp.tile([P, SGB, H1], dt.bfloat16, tag="sf")
                        nc.sync.dma_start(
                            out=sf[:, 0:nsgb, 0:H],
                            in_=selfsrc[sg[0] * P:(sg[-1] + 1) * P, :]
                            .rearrange("(b p) h -> p b h", p=P))
                        KAg = sum(kA[b] for b in sg)
                        KBg = sum(kB[b] for b in sg)
                        gab = gpool.tile([P, (capA + capB) * P], dt.bfloat16,
                                         tag="gab")
                        gva = gvb = None
                        if KAg:
                            gva = gab[:, 0:KAg * H].rearrange(
                                "p (c h) -> p c h", h=H)
                            for c0 in range(0, KAg, MAXC):
                                nch = min(MAXC, KAg - c0)
                                o = gcol + c0
                                _dma_gather_sbuf(
                                    nc, gva[:, c0:c0 + nch, :], inA,
                                    ix_t[:, o * 8:(o + nch) * 8],
                                    nch * P, H, row_bytes, self_qn[0] % NQ)
                                self_qn[0] += 1
                        if KBg:
                            gvb = gab[:, KAg * H:(KAg + KBg) * H].rearrange(
                                "p (c h) -> p c h", h=H)
                            for c0 in range(0, KBg, MAXC):
                                nch = min(MAXC, KBg - c0)
                                o = gcol + KAg + c0
                                _dma_gather_sbuf(
                                    nc, gvb[:, c0:c0 + nch, :], inB,
                                    ix_t[:, o * 8:(o + nch) * 8],
                                    nch * P, H, row_bytes, self_qn[0] % NQ)
                                self_qn[0] += 1
                        # per-block segment sums
                        aoff = 0
                        boff = 0
                        for bj, b in enumerate(sg):
                            ka, kb = kA[b], kB[b]
                            S = spool.tile([P, (ka + kb) * P], dt.float8e4,
                                           tag="s")
                            s3 = S[:].rearrange("p (c q) -> p c q", q=P)
                            ca = gcol + aoff
                            cb = gcol + KAg + boff
                            if ka:
                                nc.vector.tensor_tensor(
                                    s3[:, 0:ka, :],
                                    dl_t[:, ca:ca + ka].to_broadcast([P, ka, P]),
                                    io3.to_broadcast([P, ka, P]),
                                    op=mybir.AluOpType.is_equal)
                            if kb:
                                nc.vector.tensor_tensor(
                                    s3[:, ka:ka + kb, :],
                                    dl_t[:, cb:cb + kb].to_broadcast([P, kb, P]),
                                    io3.to_broadcast([P, kb, P]),
                                    op=mybir.AluOpType.is_equal)
                            ps = pp.tile([P, H1], dt.float32, tag="mp")
                            psv = ps[:, 0:H]
                            first = True
                            if qrow is not None:
                                nc.tensor.matmul(
                                    psv,
                                    iv_t[0:1, b * P:(b + 1) * P],
                                    qrow[0:1, :], start=True, stop=False)
                                first = False
                            # self-loop term: psv += I @ sf[b] (= table row)
                            nc.tensor.matmul(
                                psv, idn_b[:], sf[:, bj, 0:H],
                                start=first, stop=False)
                            first = False
                            nch = ka + kb
                            for c in range(ka):
                                nc.tensor.matmul(
                                    psv, s3[:, c, :], gva[:, aoff + c, 0:H],
                                    start=False,
                                    stop=(c == nch - 1))
                            for c in range(kb):
                                nc.tensor.matmul(
                                    psv, s3[:, ka + c, :], gvb[:, boff + c, 0:H],
                                    start=False,
                                    stop=(ka + c == nch - 1))
                            epilogue(b, psv)
                            aoff += ka
                            boff += kb
                        gcol += KAg + KBg

            # ---- epilogues ----
            epi1_st = {}

            def epi1(b, psv):
                g, j = b // GRPB, b % GRPB
                if j == 0:
                    epi1_st["t"] = work.tile([P, GRPB, H2], dt.bfloat16,
                                             tag="h2b", name="h2b")
                h2b = epi1_st["t"]
                tmp = work.tile([P, H1], dt.float32, tag="ep")
                d = di_t[:, b:b + 1]
                nc.scalar.activation(tmp[:], psv, Relu, scale=d)
                nc.scalar.activation(A1[:, b * P:(b + 1) * P], tmp[:], Copy,
                                     scale=d)
                # fused layer-2 feature matmul: h2 = A1_b @ W2s
                pst = pp.tile([P, P], dt.bfloat16, tag="trb")
                nc.tensor.transpose(pst[:], A1[:, b * P:(b + 1) * P], idn_b[:])
                a1T = work.tile([P, P], dt.bfloat16, tag="a1T")
                nc.scalar.activation(a1T[:], pst[:], Copy)
                ps2 = pp.tile([P, H1], dt.float32, tag="mp")
                nc.tensor.matmul(ps2[:, 0:H2], a1T[:], w2_t[:],
                                 start=True, stop=True)
                nc.scalar.activation(h2b[:, j, :], ps2[:, 0:H2], Copy)
                if j == GRPB - 1:
                    nc.sync.dma_start(
                        out=shard64[0][g * GRPB * P:(g + 1) * GRPB * P, :]
                        .rearrange("(b p) h -> p b h", p=P),
                        in_=h2b[:])

            epi2_st = {}

            def epi2(b, psv):
                g, j = b // GRPB, b % GRPB
                if j == 0:
                    epi2_st["t"] = work.tile([P, GRPB, H2], dt.bfloat16,
                                             tag="a2b", name="a2b")
                a2b = epi2_st["t"]
                tmp = work.tile([P, H2], dt.float32, tag="ep")
                d = di_t[:, b:b + 1]
                nc.scalar.activation(tmp[:], psv, Relu, scale=d)
                nc.scalar.activation(a2b[:, j, :], tmp[:], Copy, scale=d)
                if j == GRPB - 1:
                    nc.sync.dma_start(
                        out=shard64[1][g * GRPB * P:(g + 1) * GRPB * P, :]
                        .rearrange("(b p) h -> p b h", p=P),
                        in_=a2b[:])

            def epi3(b, psv):
                r = work.tile([P, H2], dt.float32, tag="ep")
                nc.scalar.activation(r[:], psv, Copy, scale=di_t[:, b:b + 1])
                pst = pp.tile([H2, P], dt.float32, tag="tr")
                nc.tensor.transpose(pst[:], r[:], idn_f[:])
                rT = work.tile([H2, P], dt.float32, tag="rT")
                nc.scalar.activation(rT[:], pst[:], Copy)
                ps3 = pp.tile([P, NCLS], dt.float32, tag="o3")
                nc.tensor.matmul(ps3[:], rT[:], w3_t[:], start=True, stop=False)
                nc.tensor.matmul(ps3[:], on_t[0:1, :], b3_t[0:1, :],
                                 start=False, stop=True)
                nc.scalar.activation(outacc[:, b * NCLS:(b + 1) * NCLS],
                                     ps3[:], Copy)

            if stages >= 3:
                mp_layer("mp1", tb1v, 0, H1, 256, q1_t, shard1, epi1)

            if stages >= 5:
                with nc.named_scope("ag2"):
                    allgather(shard64[0], full64[0])
                    fill_full(full64[0], tb64, 0)

            if stages >= 6:
                mp_layer("mp2", tb64, 0, H2, H2 * 2, q2_t, shard64[0], epi2)
            if stages >= 7:
                with nc.named_scope("ag3"):
                    allgather(shard64[1], full64[1])
                    fill_full(full64[1], tb64, RPP)
            if stages >= 8:
                mp_layer("mp3", tb64, RPP, H2, H2 * 2, None, shard64[1], epi3)

            with nc.named_scope("fin"):
                nc.sync.dma_start(out=outt.ap(), in_=outacc[:])

    nc.compile()
    return nc


def _run(inputs, trace=False, stages=99):
    x = np.asarray(inputs["x"], np.float32)
    edge_index = np.asarray(inputs["edge_index"])
    key = (hash(edge_index.tobytes()), stages)
    if key not in _cache:
        sched = _make_schedule(edge_index)
        sched["dst_all"] = np.concatenate(
            [edge_index[1], np.arange(N, dtype=np.int64)]).astype(np.int64)
        nc = _build(sched, stages=stages)
        _cache[key] = (sched, nc)
    sched, nc = _cache[key]
    sched["dst_all"] = np.concatenate(
        [edge_index[1], np.arange(N, dtype=np.int64)]).astype(np.int64)

    in_maps = _prep_inputs(
        sched, x,
        np.asarray(inputs["w1"], np.float32), np.asarray(inputs["b1"], np.float32),
        np.asarray(inputs["g1"], np.float32), np.asarray(inputs["beta1"], np.float32),
        np.asarray(inputs["m1"], np.float32), np.asarray(inputs["v1"], np.float32),
        np.asarray(inputs["w2"], np.float32), np.asarray(inputs["b2"], np.float32),
        np.asarray(inputs["g2"], np.float32), np.asarray(inputs["beta2"], np.float32),
        np.asarray(inputs["m2"], np.float32), np.asarray(inputs["v2"], np.float32),
        np.asarray(inputs["w3"], np.float32), np.asarray(inputs["b3"], np.float32),
    )
    kw = {}
    if trace:
        kw = dict(trace=True, trace_cores=[0])
    res = run_bass_kernel_spmd(nc, in_maps, core_ids=list(range(NCORES)), **kw)
    outs = []
    for c in range(NCORES):
        o = np.asarray(res.results[c]["out"])          # [P, NB*NCLS]
        outs.append(o.reshape(P, NB, NCLS).transpose(1, 0, 2).reshape(SHN, NCLS))
    out = np.concatenate(outs, axis=0)
    return out[:N].astype(np.float32), res


def kernel(**inputs) -> np.ndarray:
    out, _ = _run(inputs, trace=False)
    return out
